# revision 1
# baseline (speedup 1.0000x reference)
"""AttentiveFP pooling (PyG) distributed across 8 trn2 NeuronCores.

Sharding: nodes are split so that core k owns every node whose graph id
(batch) falls in [128*k, 128*(k+1)) -- graph-aligned shards, so no graph
straddles a core boundary.  Segment sum/max over sorted batch ids become
dense one-hot matmuls against the core-local [L,128] membership matrix,
and the per-node gather of graph quantities is the same matmul applied in
the other direction.  Cross-core reduction of the [B,H] graph tensor is a
single all_gather (shards are disjoint, so no adds are needed).  The
small GAT/GRU/Linear weights are replicated (closed over as constants).

A softmax max-subtraction is mathematically unnecessary here: within one
graph the max term is constant, so it cancels between numerator and
denominator; the raw scores are O(10), well inside fp32 exp range.
"""

import numpy as np

N, B, H, OUT, T = 200000, 1024, 256, 128, 2
NEG_SLOPE = 0.01
NCORES = 8
IDS = B // NCORES  # 128 graph ids per core

_compiled = None


def _build(L):
    import jax
    import jax.numpy as jnp
    from functools import partial

    @partial(jax.pmap, axis_name="i",
             in_axes=(0, 0, None, None, None, None, None, None, None, None,
                      None, None, None, None))
    def run(x_sh, oh, W, w_src, w_dst, bias_gat, W_ih, W_hh, b_ih, b_hh,
            W_lin, b_lin, ones_h, ones_o):
        k = jax.lax.axis_index("i")
        # pass 1: local segment sum + per-node source attention logits
        out0_l = jnp.einsum("lc,lh->ch", oh, x_sh)          # [128,H]
        a_src = x_sh @ w_src                                 # [L]
        out = jax.lax.all_gather(out0_l, "i").reshape(B, H)  # [B,H]
        for _ in range(T):
            d = out @ w_dst                                  # [B]
            d_loc = jax.lax.dynamic_slice(d, (k * IDS,), (IDS,))
            dg = oh @ d_loc                                  # [L]
            e = a_src + dg
            e = jnp.maximum(e, NEG_SLOPE * e)                # leaky_relu
            ee = jnp.exp(e)                                  # max cancels
            s_l = jnp.einsum("lc,lh->ch", oh, x_sh * ee[:, None])
            den_l = ee @ oh                                  # [128]
            s = jax.lax.all_gather(s_l, "i").reshape(B, H)
            den = jax.lax.all_gather(den_l, "i").reshape(B)
            agg = (s / den[:, None]) @ W + bias_gat
            h = jnp.where(agg > 0, agg, jnp.exp(jnp.minimum(agg, 0.0)) - 1.0)
            gi = h @ W_ih.T + b_ih
            gh = out @ W_hh.T + b_hh
            r = jax.nn.sigmoid(gi[:, :H] + gh[:, :H])
            z = jax.nn.sigmoid(gi[:, H:2 * H] + gh[:, H:2 * H])
            n = jnp.tanh(gi[:, 2 * H:] + r * gh[:, 2 * H:])
            v = (1.0 - z) * n + z * out
            out = v * jax.nn.sigmoid(v)                      # silu
        return out @ W_lin + b_lin

    return run


def kernel(x, batch, W, att_src, att_dst, bias_gat, W_ih, W_hh, b_ih, b_hh,
           W_lin, b_lin):
    global _compiled
    x = np.asarray(x, dtype=np.float32)
    batch = np.asarray(batch).astype(np.int64)

    # graph-aligned node shards: core k takes batch ids [128k, 128(k+1))
    edges = np.searchsorted(batch, np.arange(0, B + 1, IDS))
    counts = np.diff(edges)
    L = int(((counts.max() + 127) // 128) * 128)

    x_sh = np.zeros((NCORES, L, H), dtype=np.float32)
    oh = np.zeros((NCORES, L, IDS), dtype=np.float32)
    for k in range(NCORES):
        n0, n1 = int(edges[k]), int(edges[k + 1])
        c = n1 - n0
        x_sh[k, :c] = x[n0:n1]
        oh[k, np.arange(c), batch[n0:n1] - k * IDS] = 1.0

    Wf = np.asarray(W, np.float32)
    w_src = Wf @ np.asarray(att_src, np.float32)
    w_dst = Wf @ np.asarray(att_dst, np.float32)

    run = _build(L)
    res = run(x_sh, oh, Wf, w_src, w_dst,
              np.asarray(bias_gat, np.float32),
              np.asarray(W_ih, np.float32), np.asarray(W_hh, np.float32),
              np.asarray(b_ih, np.float32), np.asarray(b_hh, np.float32),
              np.asarray(W_lin, np.float32), np.asarray(b_lin, np.float32),
              np.ones((H,), np.float32), np.ones((OUT,), np.float32))
    return np.asarray(res[0])
